# revision 17
# baseline (speedup 1.0000x reference)
"""Trainium2 Bass kernel for nn_CurvStdDist (retrieval_knn).

Reference computation (per batch b, per cloud):
  x: (n,3) points, nrm: (n,3) unit normals, k=16
  idx   = 16 nearest neighbors of each point (excluding self, by squared L2)
  v     = x[idx] - x[:,None]; vhat = v / clip(||v||, 1e-12)
  kappa = mean_k |vhat . nrm|                      (n,)
  std   = std(kappa[idx], ddof=1)                  (n,)
Final: dist = mean_b ||ori_std[b] - adv_std[b] + 1e-6||_2

Sharding: 8 cores = 4 batches x 2 clouds (ori/adv); each core runs the
full n=4096 KNN pipeline for one (batch, cloud); host combines the 8
std vectors into the scalar.

Device algorithm per core:
  - -d2 row-tiles [128,4096] via K=5 fp32 matmul:
      -d2[i,j] = [2x_i, -|x_i|^2, -1] . [x_j, 1, |x_j|^2]
    plus a second PE matmul adding -1e38*I on the tile's diagonal block
    (self-exclusion), so top-16 = the 16 nearest neighbors directly.
  - top-16 per row: 2 rounds of DVE max8 / max_index / match_replace.
  - gather neighbor coords via indirect (SWDGE) DMA; kappa via DVE/ACT
    elementwise ops; kappa stored to DRAM scaled by 16 (the 1/16 mean
    factor is folded into the final sqrt scale).
  - second indirect gather of neighbor kappas; std(ddof=1) via
    mean/center/square-sum; sqrt scale 1/(15*256) undoes the 16x.
"""

import numpy as np

N = 4096          # points per cloud
P = 128           # partitions
T = N // P        # 32 row tiles
K = 16            # neighbors
BANK = 512        # psum bank width (f32)
NBANK = N // BANK
DIAG_NEG = -1.0e38   # added on the diagonal (self distance)
FILL_NEG = -3.0e38   # match_replace fill

_PROG_CACHE = {}


def _build_program(stage="full"):
    """Build + compile the single-core Bass program (shared by all 8 cores).

    stage: "mm" | "topk" | "gather" | "kappa" | "full" — debug prefixes of
    the pipeline; anything but "full" writes intermediate checksums instead.
    """
    import concourse.bacc as bacc
    import concourse.bass as bass
    import concourse.mybir as mybir
    import concourse.tile as tile

    dt = mybir.dt
    AF = mybir.ActivationFunctionType
    Alu = mybir.AluOpType

    nc = bacc.Bacc("TRN2", target_bir_lowering=False, debug=False)

    lhsT5 = nc.dram_tensor("lhsT5", [5, N], dt.float32, kind="ExternalInput")
    rhs5 = nc.dram_tensor("rhs5", [5, N], dt.float32, kind="ExternalInput")
    xyz = nc.dram_tensor("xyz", [N, 3], dt.float32, kind="ExternalInput")
    nrm = nc.dram_tensor("nrm", [N, 3], dt.float32, kind="ExternalInput")
    eye = nc.dram_tensor("eye", [P, P], dt.float32, kind="ExternalInput")
    # -1e38*I at columns 384:512 of a zero [P, 896]; slicing [384-off : 896-off]
    # yields a [P, 512] bank-row with the negative diagonal at columns off:off+P
    negpad = nc.dram_tensor("negpad", [P, 896], dt.float32, kind="ExternalInput")
    kap_d = nc.dram_tensor("kappa", [N, 1], dt.float32, kind="ExternalOutput")
    std_d = nc.dram_tensor("std", [N, 1], dt.float32, kind="ExternalOutput")

    def bcast_mid(ap, k):
        # [P, c] -> [P, k, c] with a stride-0 middle dim
        return bass.AP(ap.tensor, ap.offset, [ap.ap[0], [0, k], ap.ap[1]])

    with tile.TileContext(nc) as tc:
        with (
            tc.tile_pool(name="const", bufs=1) as constp,
            tc.tile_pool(name="srow", bufs=2) as sp,
            tc.tile_pool(name="psum", bufs=NBANK, space="PSUM") as pp,
            tc.tile_pool(name="small", bufs=3) as smp,
            tc.tile_pool(name="idxp", bufs=1) as idxp,
        ):
            lh = constp.tile_from(lhsT5.ap())
            rh = constp.tile_from(rhs5.ap())
            ey = constp.tile_from(eye.ap())
            npd = constp.tile_from(negpad.ap())
            idx_all = idxp.tile([P, T * K], dt.uint32)

            # ---------------- phase A: knn + kappa ----------------
            for t in range(T):
                S = sp.tile([P, N], dt.float32, tag="S")
                bd, off = (t * P) // BANK, (t * P) % BANK
                for b in range(NBANK):
                    ps = pp.tile([P, BANK], dt.float32, tag="ps")
                    nc.tensor.matmul(
                        out=ps[:],
                        lhsT=lh[:, t * P : (t + 1) * P],
                        rhs=rh[:, b * BANK : (b + 1) * BANK],
                        start=True,
                        stop=(b != bd),
                    )
                    if b == bd:
                        nc.tensor.matmul(
                            out=ps[:],
                            lhsT=ey[:],
                            rhs=npd[:, 384 - off : 896 - off],
                            start=False,
                            stop=True,
                        )
                    nc.scalar.copy(S[:, b * BANK : (b + 1) * BANK], ps[:])

                if stage == "mm":
                    chk = smp.tile([P, 1], dt.float32, tag="chk")
                    nc.vector.tensor_reduce(
                        chk[:], S[:], axis=mybir.AxisListType.X, op=Alu.max
                    )
                    nc.sync.dma_start(std_d.ap()[t * P : (t + 1) * P, :], chk[:])
                    continue

                i0 = idx_all[:, t * K : t * K + 8]
                i1 = idx_all[:, t * K + 8 : t * K + 16]
                vals = smp.tile([P, 16], dt.float32, tag="vals")
                nc.vector.max(vals[:, 0:8], S[:])
                nc.vector.max_index(i0, vals[:, 0:8], S[:])
                nc.vector.match_replace(S[:], vals[:, 0:8], S[:], FILL_NEG)
                nc.vector.max(vals[:, 8:16], S[:])
                nc.vector.max_index(i1, vals[:, 8:16], S[:])

                if stage == "topk":
                    chk = smp.tile([P, 1], dt.float32, tag="chk")
                    nc.vector.tensor_reduce(
                        chk[:], vals[:], axis=mybir.AxisListType.X, op=Alu.add
                    )
                    nc.sync.dma_start(std_d.ap()[t * P : (t + 1) * P, :], chk[:])
                    continue

                # gather 16 neighbor coords per point: nn[p, k*3:(k+1)*3].
                # HW indirect DMA takes ONE index per partition (contiguous
                # run per index), so issue one gather per neighbor slot.
                nn = smp.tile([P, K * 3], dt.float32, tag="nn")
                for k in range(K):
                    nc.gpsimd.indirect_dma_start(
                        out=nn[:, 3 * k : 3 * k + 3],
                        out_offset=None,
                        in_=xyz.ap(),
                        in_offset=bass.IndirectOffsetOnAxis(
                            ap=idx_all[:, t * K + k : t * K + k + 1], axis=0
                        ),
                    )
                if stage == "gather":
                    chk = smp.tile([P, 1], dt.float32, tag="chk")
                    nc.vector.tensor_reduce(
                        chk[:], nn[:], axis=mybir.AxisListType.X, op=Alu.add
                    )
                    nc.sync.dma_start(std_d.ap()[t * P : (t + 1) * P, :], chk[:])
                    continue

                xi = smp.tile([P, 3], dt.float32, tag="xi")
                nc.sync.dma_start(xi[:], xyz.ap()[t * P : (t + 1) * P, :])
                ni = smp.tile([P, 3], dt.float32, tag="ni")
                nc.sync.dma_start(ni[:], nrm.ap()[t * P : (t + 1) * P, :])

                nn3 = nn[:].rearrange("p (k c) -> p k c", c=3)
                v = smp.tile([P, K * 3], dt.float32, tag="v")
                v3 = v[:].rearrange("p (k c) -> p k c", c=3)
                nc.vector.tensor_tensor(
                    out=v3, in0=nn3, in1=bcast_mid(xi[:], K), op=Alu.subtract
                )
                vn = smp.tile([P, K * 3], dt.float32, tag="vn")
                vn3 = vn[:].rearrange("p (k c) -> p k c", c=3)
                nc.vector.tensor_tensor(
                    out=vn3, in0=v3, in1=bcast_mid(ni[:], K), op=Alu.mult
                )
                dot = smp.tile([P, K], dt.float32, tag="dot")
                nc.vector.tensor_reduce(
                    dot[:], vn3, axis=mybir.AxisListType.X, op=Alu.add
                )
                v2 = smp.tile([P, K * 3], dt.float32, tag="v2")
                v23 = v2[:].rearrange("p (k c) -> p k c", c=3)
                nc.vector.tensor_tensor(out=v23, in0=v3, in1=v3, op=Alu.mult)
                n2 = smp.tile([P, K], dt.float32, tag="n2")
                nc.vector.tensor_reduce(
                    n2[:], v23, axis=mybir.AxisListType.X, op=Alu.add
                )
                # clip ||v||^2 at 1e-24 (reference clips ||v|| at 1e-12)
                nc.vector.tensor_scalar_max(n2[:], n2[:], 1e-24)
                ri = smp.tile([P, K], dt.float32, tag="ri")
                nc.vector.reciprocal(ri[:], n2[:])
                rs = smp.tile([P, K], dt.float32, tag="rs")
                nc.scalar.activation(rs[:], ri[:], AF.Sqrt)
                sc = smp.tile([P, K], dt.float32, tag="sc")
                nc.vector.tensor_tensor(out=sc[:], in0=dot[:], in1=rs[:], op=Alu.mult)
                kap = smp.tile([P, 1], dt.float32, tag="kap")
                nc.vector.tensor_reduce(
                    kap[:],
                    sc[:],
                    axis=mybir.AxisListType.X,
                    op=Alu.add,
                    apply_absolute_value=True,
                )  # = 16 * kappa
                nc.sync.dma_start(kap_d.ap()[t * P : (t + 1) * P, :], kap[:])

            # make sure all kappa stores land before the phase-B gathers
            if stage not in ("kappa", "nobarrier"):
                tc.strict_bb_all_engine_barrier()

            # ---------------- phase B: neighbor-kappa std ----------------
            nb = (
                T
                if stage in ("full", "nobarrier", "gather2", "std1", "std2")
                else 0
            )
            for t in range(nb):
                nnk = smp.tile([P, K], dt.float32, tag="nnk")
                for k in range(K):
                    nc.gpsimd.indirect_dma_start(
                        out=nnk[:, k : k + 1],
                        out_offset=None,
                        in_=kap_d.ap(),
                        in_offset=bass.IndirectOffsetOnAxis(
                            ap=idx_all[:, t * K + k : t * K + k + 1], axis=0
                        ),
                    )
                sm = smp.tile([P, 1], dt.float32, tag="sm")
                nc.vector.tensor_reduce(
                    sm[:], nnk[:], axis=mybir.AxisListType.X, op=Alu.add
                )
                if stage == "gather2":
                    nc.sync.dma_start(std_d.ap()[t * P : (t + 1) * P, :], sm[:])
                    continue
                mn = smp.tile([P, 1], dt.float32, tag="mn")
                nc.vector.tensor_scalar_mul(mn[:], sm[:], 1.0 / K)
                cen = smp.tile([P, K], dt.float32, tag="cen")
                nc.vector.tensor_scalar(
                    out=cen[:], in0=nnk[:], scalar1=mn[:], scalar2=None,
                    op0=Alu.subtract,
                )
                if stage == "std1":
                    nc.sync.dma_start(
                        std_d.ap()[t * P : (t + 1) * P, :], cen[:, 0:1]
                    )
                    continue
                cen2 = smp.tile([P, K], dt.float32, tag="cen2")
                ss = smp.tile([P, 1], dt.float32, tag="ss")
                nc.vector.tensor_tensor(
                    out=cen2[:], in0=cen[:], in1=cen[:], op=Alu.mult
                )
                nc.vector.tensor_reduce(
                    ss[:], cen2[:], axis=mybir.AxisListType.X, op=Alu.add
                )
                if stage == "std2":
                    nc.sync.dma_start(std_d.ap()[t * P : (t + 1) * P, :], ss[:])
                    continue
                stdt = smp.tile([P, 1], dt.float32, tag="stdt")
                # std = sqrt(ss/(K-1))/K  (kappa was stored scaled by K)
                nc.scalar.activation(
                    stdt[:], ss[:], AF.Sqrt, scale=1.0 / ((K - 1) * K * K)
                )
                nc.sync.dma_start(std_d.ap()[t * P : (t + 1) * P, :], stdt[:])

    nc.compile()
    return nc


def get_program():
    if "nc" not in _PROG_CACHE:
        _PROG_CACHE["nc"] = _build_program()
    return _PROG_CACHE["nc"]


def make_in_map(x3n: np.ndarray, nrm3n: np.ndarray) -> dict:
    """Per-core inputs. x3n, nrm3n: (3, N) float32."""
    x = np.ascontiguousarray(x3n, dtype=np.float32)          # (3, N)
    xyz = np.ascontiguousarray(x.T)                          # (N, 3)
    nrm = np.ascontiguousarray(np.asarray(nrm3n, np.float32).T)
    sq = (x * x).sum(axis=0, dtype=np.float32)               # (N,)
    ones = np.ones((N,), np.float32)
    rhs5 = np.ascontiguousarray(np.stack([x[0], x[1], x[2], ones, sq]))
    lhsT5 = np.ascontiguousarray(
        np.stack([2 * x[0], 2 * x[1], 2 * x[2], -sq, -ones])
    )
    eye = np.eye(P, dtype=np.float32)
    negpad = np.zeros((P, 896), np.float32)
    negpad[:, 384:512] = np.float32(DIAG_NEG) * eye
    return {
        "lhsT5": lhsT5,
        "rhs5": rhs5,
        "xyz": xyz,
        "nrm": nrm,
        "eye": eye,
        "negpad": negpad,
    }


def combine(std_vecs: list) -> np.ndarray:
    """std_vecs: 8 arrays (N,) — cores 0-3 ori batches, 4-7 adv batches."""
    dists = []
    for b in range(4):
        diff = (
            std_vecs[b].astype(np.float64)
            - std_vecs[4 + b].astype(np.float64)
            + 1e-6
        )
        dists.append(np.sqrt((diff * diff).sum()))
    return np.asarray(np.mean(dists), dtype=np.float32)


def kernel(ori_data, adv_data, ori_normal):
    from concourse.bass_utils import run_bass_kernel_spmd

    nc = get_program()
    in_maps = []
    for cloud in (ori_data, adv_data):
        for b in range(4):
            in_maps.append(make_in_map(cloud[b], ori_normal[b]))
    res = run_bass_kernel_spmd(nc, in_maps, core_ids=list(range(8)))
    std_vecs = [r["std"][:, 0] for r in res.results]
    return combine(std_vecs)
